# revision 1
# baseline (speedup 1.0000x reference)
"""AttentionLSTM cell — data-parallel over batch across 8 NeuronCores.

B=32, T=128, D=512, U=512. The sequential scan prevents sequence
parallelism; per-step GEMMs stay local (weights replicated), so each core
runs 4 independent sequences. Inputs are sharded on the host, the scan
runs on-device via PJRT on all 8 cores, outputs are concatenated.
"""

import numpy as np
import jax
import jax.numpy as jnp
from functools import partial

B, T, D, U = 32, 128, 512, 512
N_CORES = 8


def _hard_sigmoid(z):
    return jnp.clip(0.2 * z + 0.5, 0.0, 1.0)


@partial(jax.pmap, axis_name="i",
         in_axes=(0, None, None, None, None, None, None, None, None))
def _run_shard(x, kernel, recurrent_kernel, attention_kernel,
               attention_W, attention_U, attention_V, bias, attention_b):
    u = recurrent_kernel.shape[0]
    # Hoisted time-invariant projections
    att_x = jnp.einsum("btd,du->btu", x, attention_W) + attention_b
    xk = jnp.einsum("btd,dk->btk", x, kernel) + bias

    def step(carry, xk_t):
        h, c = carry
        e = jnp.tanh(att_x + (h @ attention_U)[:, None, :])
        scores = jnp.einsum("btu,uo->bt", e, attention_V)
        alpha = jax.nn.softmax(scores, axis=1)
        z = jnp.einsum("bt,btd->bd", alpha, x)
        gates = xk_t + h @ recurrent_kernel + z @ attention_kernel
        i = _hard_sigmoid(gates[:, :u])
        f = _hard_sigmoid(gates[:, u:2 * u])
        c_new = f * c + i * jnp.tanh(gates[:, 2 * u:3 * u])
        o = _hard_sigmoid(gates[:, 3 * u:])
        h_new = o * jnp.tanh(c_new)
        return (h_new, c_new), h_new

    b_local = x.shape[0]
    h0 = jnp.zeros((b_local, u), x.dtype)
    c0 = jnp.zeros((b_local, u), x.dtype)
    (_, _), hs = jax.lax.scan(step, (h0, c0), jnp.swapaxes(xk, 0, 1))
    return jnp.swapaxes(hs, 0, 1)


def kernel(**inputs):
    x = np.asarray(inputs["x"], np.float32)
    xs = x.reshape(N_CORES, B // N_CORES, T, D)
    args = tuple(
        jnp.asarray(inputs[k], jnp.float32)
        for k in ("kernel", "recurrent_kernel", "attention_kernel",
                  "attention_W", "attention_U", "attention_V",
                  "bias", "attention_b"))
    out = _run_shard(jnp.asarray(xs), *args)
    out = np.asarray(jax.device_get(out), np.float32)
    return out.reshape(B, T, U)
